# revision 49
# baseline (speedup 1.0000x reference)
"""Trainium2 Bass kernel for nn_Attention_12266426598027.

GQA attention layer (B=4, S=2048, H=896, 14 q-heads / 2 kv-heads, HD=64,
RoPE theta=1e6, causal) distributed over 8 NeuronCores.

Sharding: core = (batch b, kv-group g). Each core computes 7 q-heads against
its kv head for one batch, including its slice of the QKV projection and a
partial o_proj (448 of the 896 contraction dims). The two partial o_proj
outputs per batch are summed on the host.

v2 design notes (vs the 500us baseline):
- Everything is emitted as ONE dense PE stream: QKV projection units (7
  accumulating matmuls each) are interleaved into the first q-chunk of the
  attention, o_proj units into subsequent q-chunks, so the PE never idles
  >3.4us and the HAM clock stays at 2.4 GHz.
- Loop order is q-chunk (j) OUTER, head-pair inner; o_proj for chunk j runs
  during attention of chunk j+1.
- x/w/q/k/v/cos/sin/probs are bf16 (rel-err budget ~2e-3 << 2e-2 tol):
  halves SBUF + DVE RoPE time, enables FWL weight loads. Scores PSUM stays
  fp32 (TRN2 requirement).
- Scores are trimmed to the causal range (diag chunk t computes only
  512-128t q-cols), causal masking is done by GpSimd affine_select ZEROING
  on the bf16 probs after exp, not DVE adds on PSUM.
- Softmax row sums come from the ones-column appended to V (PV matmul M=65);
  the 1/rowsum uses reciprocal_approx_fast (single DVE op, ~51 ULP) instead
  of the 8-cycle/elem iterative reciprocal that dominated the baseline DVE.
- Scores pipeline: 2 head streams x 1 buf [128,1024] fp32 PSUM tiles keep
  the ACT (exp) engine -- the binding engine at ~130us -- saturated. PV lags
  scores by one group in the emission order so the PE FIFO never head-blocks.
"""
import sys

for _p in ('/opt/trn_rl_repo', '/root/.axon_site'):
    if _p not in sys.path:
        sys.path.insert(0, _p)

import numpy as np

B, S, H = 4, 2048, 896
NH, NKV, HD = 14, 2, 64
NHC, DQ = 7, 448          # q-heads per core, their stacked dim
ROPE_THETA = 1e6
M_SIZES = [128, 128, 128, 128, 128]  # m-tiles over 640 = 448q+64k+64ksw+64v
M_OFFS = [0, 128, 256, 384, 512]

_PROGRAM_CACHE = {}


def _build_program():
    from collections import deque
    import concourse.bass as bass
    from concourse import bacc
    import concourse.mybir as mybir
    import concourse.tile as tile
    F32 = mybir.dt.float32
    F32R = mybir.dt.float32r
    BF16 = mybir.dt.bfloat16
    ALU = mybir.AluOpType
    AF = mybir.ActivationFunctionType

    nc = bacc.Bacc("TRN2", target_bir_lowering=False, debug=False, num_devices=8)

    xT_d = nc.dram_tensor("xT", [H, S], BF16, kind="ExternalInput").ap()
    wT_d = nc.dram_tensor("wT", [H, 640], BF16, kind="ExternalInput").ap()
    bias_d = nc.dram_tensor("bias", [640], F32, kind="ExternalInput").ap()
    woT_d = nc.dram_tensor("woT", [DQ, H], BF16, kind="ExternalInput").ap()
    cos2_d = nc.dram_tensor("cos2", [128, S], BF16, kind="ExternalInput").ap()
    sinm2_d = nc.dram_tensor("sinm2", [128, S], BF16, kind="ExternalInput").ap()
    ident_d = nc.dram_tensor("ident64", [64, 64], BF16, kind="ExternalInput").ap()
    yT_d = nc.dram_tensor("yT", [H, S], F32, kind="ExternalOutput").ap()
    import os as _os
    DEBUG = _os.environ.get("KERNEL_DEBUG_OUTPUTS", "0") == "1"
    if DEBUG:
        dbg = {}
        for nm, shp, dt in [("dqkv", [5 * 128, S], BF16),
                            ("dk2", [128, S], BF16),
                            ("dq6d", [128, S], BF16),
                            ("dv", [16 * 128, 65], BF16),
                            ("dpr", [7 * 128, 1280], BF16),
                            ("drcp", [28, 512], F32),
                            ("dattn", [448, 2048], BF16)]:
            dbg[nm] = nc.dram_tensor(nm, shp, dt, kind="ExternalOutput").ap()

    with tile.TileContext(nc) as tc:
        with tc.tile_pool(name="persist", bufs=1) as pp, \
             tc.tile_pool(name="work", bufs=1) as pw, \
             tc.tile_pool(name="psum", bufs=1, space="PSUM") as ps:

            # ---- persistent SBUF ------------------------------------------
            qkv = [pp.tile([128, S], BF16, tag=f"qkv{m}", name=f"qkv{m}")
                   for m in range(5)]
            k2 = pp.tile([128, S], BF16, tag="k2", name="k2")
            q6d = pp.tile([128, S], BF16, tag="q6d", name="q6d")
            v_sb = [pp.tile([128, 65], BF16, tag=f"v{i}", name=f"v{i}")
                    for i in range(16)]
            xt = [pp.tile([128, S], BF16, tag=f"x{i}", name=f"x{i}")
                  for i in range(7)]
            wt = [pp.tile([128, 640], BF16, tag=f"w{i}", name=f"w{i}")
                  for i in range(7)]
            wo = [pp.tile([128, H], BF16, tag=f"wo{i}", name=f"wo{i}")
                  for i in range(4)]
            cos2t = pp.tile([128, S], BF16, tag="cos2t", name="cos2t")
            sinm2t = pp.tile([128, S], BF16, tag="sinm2t", name="sinm2t")
            biast = pp.tile([128, 5], F32, tag="biast", name="biast")
            ident = pp.tile([128, 64], BF16, tag="ident", name="ident")
            warm = pp.tile([128, 1], F32, tag="warm", name="warm")

            # ---- input DMAs -----------------------------------------------
            nc.sync.dma_start(biast[:], bias_d.rearrange("(m p) -> p m", p=128))
            # identity block at partitions 64:128 to transpose the V rows of
            # the [ksw | v] m4 tile
            nc.sync.dma_start(ident[64:128, :], ident_d[:])
            # DMA order: pair wt[h] with xt[h] slice 0 so the first QKV unit
            # can start after ~2 transfers; later slices follow per-window
            for i in range(7):
                nc.sync.dma_start(wt[i][:], wT_d[128 * i:128 * i + 128, :])
                nc.sync.dma_start(xt[i][:, 0:512],
                                  xT_d[128 * i:128 * i + 128, 0:512])
            nc.sync.dma_start(cos2t[:, 0:512], cos2_d[:, 0:512])
            nc.sync.dma_start(sinm2t[:, 0:512], sinm2_d[:, 0:512])
            def load_wave(sc):
                # input DMAs for q-window sc, deferred so the DMA queues
                # never have a deep backlog ahead of the small latency-
                # critical internal moves (xsw/k2)
                for i in range(7):
                    nc.sync.dma_start(
                        xt[i][:, 512 * sc:512 * sc + 512],
                        xT_d[128 * i:128 * i + 128, 512 * sc:512 * sc + 512])
                nc.sync.dma_start(cos2t[:, 512 * sc:512 * sc + 512],
                                  cos2_d[:, 512 * sc:512 * sc + 512])
                nc.sync.dma_start(sinm2t[:, 512 * sc:512 * sc + 512],
                                  sinm2_d[:, 512 * sc:512 * sc + 512])
                if sc == 3:
                    for cc in range(4):
                        K = 128 if cc < 3 else 64
                        nc.sync.dma_start(wo[cc][0:K, :],
                                          woT_d[128 * cc:128 * cc + K, :])
            # pre-load the exp table set during the QKV prefix
            nc.scalar.activation(warm[:], biast[:, 0:1], AF.Exp,
                                 bias=0.0, scale=0.0)

            # ---- QKV projection + RoPE emission helpers -------------------
            def qkv_unit(m, sc):
                M, mo = M_SIZES[m], M_OFFS[m]
                t = ps.tile([128, 512], F32, tag="aux", bufs=2,
                            name=f"qkvps{m}_{sc}")
                for h in range(7):
                    nc.tensor.matmul(
                        t[0:M, :], wt[h][:, mo:mo + M],
                        xt[h][:, 512 * sc:512 * sc + 512],
                        start=(h == 0), stop=(h == 6))
                nc.vector.tensor_scalar_add(
                    qkv[m][0:M, 512 * sc:512 * sc + 512], t[0:M, :],
                    biast[0:M, m:m + 1])

            def rope_chunk(m, c):
                # RoPE on a [128, 512] column window of qkv[m].  Window-0
                # moves go out on the (idle) scalar-engine DMA trigger so
                # they don't queue behind the bulk input loads.
                eng = nc.sync
                cs = slice(512 * c, 512 * c + 512)
                xsw = pw.tile([128, 512], BF16, tag="xsw", bufs=2,
                              name=f"xsw{m}_{c}")
                eng.dma_start(xsw[0:32, :], qkv[m][32:64, cs])
                eng.dma_start(xsw[32:64, :], qkv[m][0:32, cs])
                eng.dma_start(xsw[64:96, :], qkv[m][96:128, cs])
                eng.dma_start(xsw[96:128, :], qkv[m][64:96, cs])
                tsin = pw.tile([128, 512], BF16, tag="tsin", bufs=2,
                               name=f"tsin{m}_{c}")
                nc.vector.tensor_tensor(tsin[:], xsw[:], sinm2t[:, cs],
                                        ALU.mult)
                nc.vector.tensor_tensor(qkv[m][:, cs], qkv[m][:, cs],
                                        cos2t[:, cs], ALU.mult)
                nc.vector.tensor_tensor(qkv[m][:, cs], qkv[m][:, cs],
                                        tsin[:], ALU.add)

            def kv_chunk(c):
                # K RoPE straight into both k2 halves as pure DVE tensor ops:
                # the swapped K comes pre-projected in qkv[4][0:64] (host
                # appended Wk[perm] rows), so no partition-move DMAs sit on
                # the critical path to the first scores.
                cs = slice(512 * c, 512 * c + 512)
                tk = pw.tile([128, 512], BF16, tag="tk", bufs=2,
                             name=f"tk{c}")
                nc.vector.tensor_tensor(tk[0:64, :], qkv[4][0:64, cs],
                                        sinm2t[0:64, cs], ALU.mult)
                nc.vector.tensor_tensor(tk[64:128, :], qkv[4][0:64, cs],
                                        sinm2t[0:64, cs], ALU.mult)
                nc.vector.tensor_tensor(k2[0:64, cs], qkv[3][64:128, cs],
                                        cos2t[64:128, cs], ALU.mult)
                nc.vector.tensor_tensor(k2[64:128, cs], qkv[3][64:128, cs],
                                        cos2t[64:128, cs], ALU.mult)
                nc.vector.tensor_tensor(k2[0:64, cs], k2[0:64, cs],
                                        tk[0:64, :], ALU.add)
                nc.vector.tensor_tensor(k2[64:128, cs], k2[64:128, cs],
                                        tk[64:128, :], ALU.add)
                # head-6 q RoPE (rows 0:64 of m3) + its row-64:128 copy
                xsw = pw.tile([64, 512], BF16, tag="xsw6", bufs=2,
                              name=f"xsw6_{c}")
                nc.sync.dma_start(xsw[0:32, :], qkv[3][32:64, cs])
                nc.sync.dma_start(xsw[32:64, :], qkv[3][0:32, cs])
                tsin = pw.tile([64, 512], BF16, tag="tsin6", bufs=2,
                               name=f"tsin6_{c}")
                nc.vector.tensor_tensor(tsin[:], xsw[:], sinm2t[0:64, cs],
                                        ALU.mult)
                nc.vector.tensor_tensor(qkv[3][0:64, cs], qkv[3][0:64, cs],
                                        cos2t[0:64, cs], ALU.mult)
                nc.vector.tensor_tensor(qkv[3][0:64, cs], qkv[3][0:64, cs],
                                        tsin[:], ALU.add)
                nc.sync.dma_start(q6d[64:128, cs], qkv[3][0:64, cs])

            def v_chunk(c):
                for i in range(4 * c, 4 * c + 4):
                    t = ps.tile([128, 64], BF16, tag="aux", bufs=2,
                                name=f"vtr{i}")
                    nc.tensor.transpose(
                        t[:], qkv[4][64:128, 128 * i:128 * i + 128],
                        ident[64:128, :])
                    nc.vector.tensor_copy(v_sb[i][:, 0:64], t[:])
                    nc.gpsimd.memset(v_sb[i][:, 64:65], 1.0)

            # ---- prefix: only window 0 of K and V, so attention j=0 can
            # start after ~2 QKV units ---------------------------------------
            qkv_unit(3, 0)
            qkv_unit(4, 0)
            kv_chunk(0)
            v_chunk(0)

            # ---- filler queue for dense PE stream: the rest of QKV + RoPE,
            # chunk-major so window 0 of every q-tile lands first -----------
            fillers = deque()
            emitted = {}
            for m in (0, 1, 2):
                fillers.append((None, lambda m=m: qkv_unit(m, 0)))
                fillers.append((("r", m, 0), lambda m=m: rope_chunk(m, 0)))
            for c in range(1, 4):
                fillers.append((None, lambda c=c: load_wave(c)))
                fillers.append((None, lambda c=c: qkv_unit(3, c)))
                fillers.append((None, lambda c=c: qkv_unit(4, c)))
                fillers.append((("kv", c), lambda c=c: kv_chunk(c)))
                fillers.append((("v", c), lambda c=c: v_chunk(c)))
                for m in (0, 1, 2):
                    fillers.append((None, lambda m=m, c=c: qkv_unit(m, c)))
                    fillers.append((("r", m, c),
                                    lambda m=m, c=c: rope_chunk(m, c)))

            def _pop_one():
                key, fn = fillers.popleft()
                fn()
                if key is not None:
                    emitted[key] = True

            def pop_fillers(n):
                for _ in range(n):
                    if fillers:
                        _pop_one()

            def ensure(key):
                while fillers and not emitted.get(key, False):
                    _pop_one()

            emitted[("kv", 0)] = emitted[("v", 0)] = True

            # ---- attention ------------------------------------------------
            # group = (chunks, widths): full pairs then diagA, diagB
            def groups_for(j):
                gs = []
                for c0 in range(0, 4 * j, 2):
                    gs.append(([c0, c0 + 1], [512, 512]))
                gs.append(([4 * j, 4 * j + 1], [512, 384]))
                gs.append(([4 * j + 2, 4 * j + 3], [256, 128]))
                return gs

            attn = {}   # (hp, j) -> SBUF tile holding normalized attnT
            HP_ORDER = [3, 0, 1, 2]
            dbg_rcp_row = [0]

            def scores_lhs_rhs(hp, h, c, qs):
                # returns (lhsT, rhs) for scores matmul of head h, chunk c
                cs = slice(128 * c, 128 * c + 128)
                if hp < 3:
                    if h % 2 == 0:
                        return k2[0:64, cs], qkv[hp][0:64, qs]
                    return k2[64:128, cs], qkv[hp][64:128, qs]
                # head 6: alternate row groups by chunk parity for PE overlap
                if c % 2 == 0:
                    return k2[0:64, cs], qkv[3][0:64, qs]
                return k2[64:128, cs], q6d[64:128, qs]

            for j in range(4):
                gs = groups_for(j)
                nkc = 4 * j + 4
                ensure(("kv", j))
                ensure(("v", j))
                for hp in HP_ORDER:
                    if hp < 3:
                        ensure(("r", hp, j))
                    heads = [2 * hp, 2 * hp + 1] if hp < 3 else [6]
                    pv = {h: ps.tile([65, 512], F32, tag=f"pv{h % 2}",
                                     name=f"pv{hp}_{j}_{h}")
                          for h in heads}
                    pending = None
                    for gi, (chunks, widths) in enumerate(gs):
                        W = sum(widths)
                        offs = [0, widths[0]]
                        scts, prs = {}, {}
                        for h in heads:
                            strm = (h % 2) if hp < 3 else (gi % 2)
                            sct = ps.tile([128, W], F32, tag=f"sc{strm}",
                                          name=f"sc{hp}_{j}_{gi}_{h}")
                            scts[h] = sct
                            for i, c in enumerate(chunks):
                                w = widths[i]
                                qs = slice(512 * j + 512 - w, 512 * j + 512)
                                if hp == 3 and gi >= len(gs) - 2:
                                    # solo head diag groups: keep on row
                                    # group 0 (q6d copy may lag at j=0)
                                    cs = slice(128 * c, 128 * c + 128)
                                    lhs, rhs = k2[0:64, cs], qkv[3][0:64, qs]
                                else:
                                    lhs, rhs = scores_lhs_rhs(hp, h, c, qs)
                                nc.tensor.matmul(
                                    sct[:, offs[i]:offs[i] + w], lhs, rhs,
                                    start=True, stop=True)
                        for h in heads:
                            strm = (h % 2) if hp < 3 else (gi % 2)
                            pt = pw.tile([128, W], BF16, tag=f"pr{strm}",
                                         bufs=3, name=f"pr{hp}_{j}_{gi}_{h}")
                            prs[h] = pt
                            nc.scalar.activation(pt[:, 0:W], scts[h][:, 0:W],
                                                 AF.Exp, bias=0.0, scale=0.125)
                            # zero the above-diagonal triangles of diag chunks
                            for i, c in enumerate(chunks):
                                t = c - 4 * j
                                if t >= 0:
                                    sl = pt[:, offs[i]:offs[i] + 128]
                                    nc.gpsimd.affine_select(
                                        out=sl, in_=sl, compare_op=ALU.is_ge,
                                        fill=0.0, base=0, pattern=[[1, 128]],
                                        channel_multiplier=-1)
                            if DEBUG and j == 0:
                                h_ = heads.index(h) if hp == 3 else h
                                co = 0 if gi == len(gs) - 2 else 896
                                nc.sync.dma_start(
                                    dbg["dpr"][128 * h:128 * h + 128,
                                               co:co + W], pt[:, 0:W])
                        if pending is not None:
                            pending()
                        def make_pv(chunks=chunks, widths=widths, offs=offs,
                                    prs=prs):
                            for h in heads:
                                for i, c in enumerate(chunks):
                                    w = widths[i]
                                    nc.tensor.matmul(
                                        pv[h][:, 512 - w:512], v_sb[c][:],
                                        prs[h][:, offs[i]:offs[i] + w],
                                        start=(c == 0), stop=(c == nkc - 1))
                        pending = make_pv
                        pop_fillers(1)
                    pending()
                    # normalize: evacuate pv to SBUF promptly (frees the PSUM
                    # bank for the next head pair), then 1/rowsum via fast
                    # approx + broadcast + in-place scale on SBUF
                    for h in heads:
                        if (hp, j) not in attn:
                            P = 128 if hp < 3 else 64
                            attn[(hp, j)] = pw.tile(
                                [P, 512], BF16, tag=f"attn{hp}", bufs=2,
                                name=f"attn{hp}_{j}")
                        dst = attn[(hp, j)][64 * (h % 2):64 * (h % 2) + 64, :]
                        au = pw.tile([64, 512], BF16, tag=f"au{h % 2}",
                                     bufs=2, name=f"au{hp}_{j}_{h}")
                        rs = pw.tile([1, 512], F32, tag="rs", bufs=2,
                                     name=f"rs{hp}_{j}_{h}")
                        # evacuate pv PSUM with high scheduler priority: the
                        # bank is the next head-pair's PV accumulator
                        with tc.high_priority():
                            nc.vector.tensor_copy(au[:], pv[h][0:64, :])
                            # custom-DVE ops drop the input partition offset,
                            # so stage the rowsum row to partition 0 first
                            nc.vector.tensor_copy(rs[:], pv[h][64:65, :])
                        rcp = pw.tile([1, 512], F32, tag="rcp", bufs=2,
                                      name=f"rcp{hp}_{j}_{h}")
                        nc.vector.reciprocal_approx_fast(rcp[:], rs[:])
                        if DEBUG:
                            r = dbg_rcp_row[0]
                            dbg_rcp_row[0] += 1
                            nc.sync.dma_start(dbg["drcp"][r:r + 1, :], rcp[:])
                        rb = pw.tile([64, 512], F32, tag="rb", bufs=2,
                                     name=f"rb{hp}_{j}_{h}")
                        nc.gpsimd.partition_broadcast(rb[:], rcp[:])
                        nc.vector.tensor_tensor(dst, au[:], rb[:], ALU.mult)
                    if DEBUG:
                        P = 128 if hp < 3 else 64
                        nc.sync.dma_start(
                            dbg["dattn"][128 * hp:128 * hp + P,
                                         512 * j:512 * j + 512],
                            attn[(hp, j)][0:P, :])
                # queue o_proj units for this j as fillers for the next j
                def oproj_unit(j=j, ot=0):
                    pys = ps.tile([128, 512], F32, tag="aux", bufs=2,
                                  name=f"py{j}_{ot}")
                    for cc in range(4):
                        K = 128 if cc < 3 else 64
                        nc.tensor.matmul(
                            pys[:], wo[cc][0:K, 128 * ot:128 * ot + 128],
                            attn[(cc, j)][0:K, :],
                            start=(cc == 0), stop=(cc == 3))
                    osb = pw.tile([128, 512], F32, tag="osb", bufs=2,
                                  name=f"osb{j}_{ot}")
                    with tc.high_priority():
                        nc.vector.tensor_copy(osb[:], pys[:])
                    nc.sync.dma_start(
                        yT_d[128 * ot:128 * ot + 128,
                             512 * j:512 * j + 512], osb[:])
                for ot in range(7):
                    fillers.append((None, lambda j=j, ot=ot: oproj_unit(j, ot)))
            # flush remaining o_proj units (last j's)
            while fillers:
                fillers.popleft()[1]()

            if DEBUG:
                for m in range(5):
                    nc.sync.dma_start(dbg["dqkv"][128 * m:128 * m + 128, :],
                                      qkv[m][:])
                nc.sync.dma_start(dbg["dk2"][:], k2[:])
                nc.sync.dma_start(dbg["dq6d"][:], q6d[:])
                for i in range(16):
                    nc.sync.dma_start(dbg["dv"][128 * i:128 * i + 128, :],
                                      v_sb[i][:])

    nc.compile()
    return nc


def _host_prep(inputs):
    import ml_dtypes
    bf16 = ml_dtypes.bfloat16
    hid = np.ascontiguousarray(np.asarray(inputs["hidden_states"], np.float32))
    pos = np.asarray(inputs["position_ids"])[0].astype(np.float32)
    Wq = np.asarray(inputs["Wq"], np.float32)
    bq = np.asarray(inputs["bq"], np.float32)
    Wk = np.asarray(inputs["Wk"], np.float32)
    bk = np.asarray(inputs["bk"], np.float32)
    Wv = np.asarray(inputs["Wv"], np.float32)
    bv = np.asarray(inputs["bv"], np.float32)
    Wo = np.asarray(inputs["Wo"], np.float32)

    inv = (1.0 / (ROPE_THETA ** (np.arange(0, HD, 2, dtype=np.float32) / HD))
           ).astype(np.float32)
    freqs = pos[:, None] * inv[None, :]
    emb = np.concatenate([freqs, freqs], -1)            # [S, 64]
    cosT = np.cos(emb).T.astype(np.float32)             # [64, S]
    sinT = np.sin(emb).T.astype(np.float32)
    sinm = sinT.copy()
    sinm[0:32] *= -1.0                                  # fold rotate_half sign
    cos2 = np.ascontiguousarray(np.vstack([cosT, cosT])).astype(bf16)
    sinm2 = np.ascontiguousarray(np.vstack([sinm, sinm])).astype(bf16)

    # rotate_half partner permutation for the pre-swapped K projection
    perm = np.concatenate([np.arange(32, 64), np.arange(0, 32)])
    maps = []
    for b in range(B):
        for g in range(2):
            xT = np.ascontiguousarray(hid[b].T).astype(bf16)
            Wkg = Wk[64 * g:64 * g + 64]
            bkg = bk[64 * g:64 * g + 64]
            Wsl = np.concatenate([Wq[448 * g:448 * g + 448],
                                  Wkg, Wkg[perm],
                                  Wv[64 * g:64 * g + 64]], 0)
            wT = np.ascontiguousarray(Wsl.T).astype(bf16)  # [896, 640]
            bias = np.concatenate([bq[448 * g:448 * g + 448],
                                   bkg, bkg[perm],
                                   bv[64 * g:64 * g + 64]]).astype(np.float32)
            woT = np.ascontiguousarray(Wo[:, 448 * g:448 * g + 448].T
                                       ).astype(bf16)
            maps.append(dict(xT=xT, wT=wT, bias=bias, woT=woT,
                             cos2=cos2, sinm2=sinm2,
                             ident64=np.eye(64, dtype=bf16)))
    return maps


def kernel(**inputs) -> np.ndarray:
    from concourse.bass_utils import run_bass_kernel_spmd

    if "nc" not in _PROGRAM_CACHE:
        _PROGRAM_CACHE["nc"] = _build_program()
    nc = _PROGRAM_CACHE["nc"]

    in_maps = _host_prep(inputs)
    res = run_bass_kernel_spmd(nc, in_maps, core_ids=list(range(8)),
                               **_PROGRAM_CACHE.get("run_kwargs", {}))
    _PROGRAM_CACHE["last_result"] = res
    yTs = [res.results[i]["yT"] for i in range(8)]
    out = np.stack([(yTs[2 * b] + yTs[2 * b + 1]).T for b in range(B)], 0)
    return np.ascontiguousarray(out)


# revision 50
# speedup vs baseline: 1.2464x; 1.2464x over previous
"""Trainium2 Bass kernel for nn_Attention_12266426598027.

GQA attention layer (B=4, S=2048, H=896, 14 q-heads / 2 kv-heads, HD=64,
RoPE theta=1e6, causal) distributed over 8 NeuronCores.

Sharding: core = (batch b, kv-group g). Each core computes 7 q-heads against
its kv head for one batch, including its slice of the QKV projection and a
partial o_proj (448 of the 896 contraction dims). The two partial o_proj
outputs per batch are summed on the host.

v2 design notes (vs the 500us baseline):
- Everything is emitted as ONE dense PE stream: QKV projection units (7
  accumulating matmuls each) are interleaved into the first q-chunk of the
  attention, o_proj units into subsequent q-chunks, so the PE never idles
  >3.4us and the HAM clock stays at 2.4 GHz.
- Loop order is q-chunk (j) OUTER, head-pair inner; o_proj for chunk j runs
  during attention of chunk j+1.
- x/w/q/k/v/cos/sin/probs are bf16 (rel-err budget ~2e-3 << 2e-2 tol):
  halves SBUF + DVE RoPE time, enables FWL weight loads. Scores PSUM stays
  fp32 (TRN2 requirement).
- Scores are trimmed to the causal range (diag chunk t computes only
  512-128t q-cols), causal masking is done by GpSimd affine_select ZEROING
  on the bf16 probs after exp, not DVE adds on PSUM.
- Softmax row sums come from the ones-column appended to V (PV matmul M=65);
  the 1/rowsum uses reciprocal_approx_fast (single DVE op, ~51 ULP) instead
  of the 8-cycle/elem iterative reciprocal that dominated the baseline DVE.
- Scores pipeline: 2 head streams x 1 buf [128,1024] fp32 PSUM tiles keep
  the ACT (exp) engine -- the binding engine at ~130us -- saturated. PV lags
  scores by one group in the emission order so the PE FIFO never head-blocks.
"""
import sys

for _p in ('/opt/trn_rl_repo', '/root/.axon_site'):
    if _p not in sys.path:
        sys.path.insert(0, _p)

import numpy as np

B, S, H = 4, 2048, 896
NH, NKV, HD = 14, 2, 64
NHC, DQ = 7, 448          # q-heads per core, their stacked dim
ROPE_THETA = 1e6
M_SIZES = [128, 128, 128, 128, 64]   # qkv m-tiles over 576 = 448q + 64k + 64v
M_OFFS = [0, 128, 256, 384, 512]

_PROGRAM_CACHE = {}


def _build_program():
    from collections import deque
    import concourse.bass as bass
    from concourse import bacc
    import concourse.mybir as mybir
    import concourse.tile as tile
    F32 = mybir.dt.float32
    F32R = mybir.dt.float32r
    BF16 = mybir.dt.bfloat16
    ALU = mybir.AluOpType
    AF = mybir.ActivationFunctionType

    nc = bacc.Bacc("TRN2", target_bir_lowering=False, debug=False, num_devices=8)

    xT_d = nc.dram_tensor("xT", [H, S], BF16, kind="ExternalInput").ap()
    wT_d = nc.dram_tensor("wT", [H, 576], BF16, kind="ExternalInput").ap()
    bias_d = nc.dram_tensor("bias", [640], F32, kind="ExternalInput").ap()
    woT_d = nc.dram_tensor("woT", [DQ, H], BF16, kind="ExternalInput").ap()
    cos2_d = nc.dram_tensor("cos2", [128, S], BF16, kind="ExternalInput").ap()
    sinm2_d = nc.dram_tensor("sinm2", [128, S], BF16, kind="ExternalInput").ap()
    ident_d = nc.dram_tensor("ident64", [64, 64], BF16, kind="ExternalInput").ap()
    yT_d = nc.dram_tensor("yT", [H, S], F32, kind="ExternalOutput").ap()
    import os as _os
    DEBUG = _os.environ.get("KERNEL_DEBUG_OUTPUTS", "0") == "1"
    if DEBUG:
        dbg = {}
        for nm, shp, dt in [("dqkv", [5 * 128, S], BF16),
                            ("dk2", [128, S], BF16),
                            ("dq6d", [128, S], BF16),
                            ("dv", [16 * 128, 65], BF16),
                            ("dpr", [7 * 128, 1280], BF16),
                            ("drcp", [28, 512], F32),
                            ("dattn", [448, 2048], BF16)]:
            dbg[nm] = nc.dram_tensor(nm, shp, dt, kind="ExternalOutput").ap()

    with tile.TileContext(nc) as tc:
        with tc.tile_pool(name="persist", bufs=1) as pp, \
             tc.tile_pool(name="work", bufs=1) as pw, \
             tc.tile_pool(name="psum", bufs=1, space="PSUM") as ps:

            # ---- persistent SBUF ------------------------------------------
            qkv = [pp.tile([128, S], BF16, tag=f"qkv{m}", name=f"qkv{m}")
                   for m in range(5)]
            k2 = pp.tile([128, S], BF16, tag="k2", name="k2")
            q6d = pp.tile([128, S], BF16, tag="q6d", name="q6d")
            v_sb = [pp.tile([128, 65], BF16, tag=f"v{i}", name=f"v{i}")
                    for i in range(16)]
            xt = [pp.tile([128, S], BF16, tag=f"x{i}", name=f"x{i}")
                  for i in range(7)]
            wt = [pp.tile([128, 576], BF16, tag=f"w{i}", name=f"w{i}")
                  for i in range(7)]
            wo = [pp.tile([128, H], BF16, tag=f"wo{i}", name=f"wo{i}")
                  for i in range(4)]
            cos2t = pp.tile([128, S], BF16, tag="cos2t", name="cos2t")
            sinm2t = pp.tile([128, S], BF16, tag="sinm2t", name="sinm2t")
            biast = pp.tile([128, 5], F32, tag="biast", name="biast")
            ident = pp.tile([64, 64], BF16, tag="ident", name="ident")
            warm = pp.tile([128, 1], F32, tag="warm", name="warm")

            # ---- input DMAs -----------------------------------------------
            nc.sync.dma_start(biast[:], bias_d.rearrange("(m p) -> p m", p=128))
            nc.sync.dma_start(ident[:], ident_d[:])
            # DMA order: pair wt[h] with xt[h] slice 0 so the first QKV unit
            # can start after ~2 transfers; later slices follow per-window
            for i in range(7):
                nc.sync.dma_start(wt[i][:], wT_d[128 * i:128 * i + 128, :])
                nc.sync.dma_start(xt[i][:, 0:512],
                                  xT_d[128 * i:128 * i + 128, 0:512])
            nc.sync.dma_start(cos2t[:, 0:512], cos2_d[:, 0:512])
            nc.sync.dma_start(sinm2t[:, 0:512], sinm2_d[:, 0:512])
            def load_wave(sc):
                # input DMAs for q-window sc, deferred so the DMA queues
                # never have a deep backlog ahead of the small latency-
                # critical internal moves (xsw/k2)
                for i in range(7):
                    nc.sync.dma_start(
                        xt[i][:, 512 * sc:512 * sc + 512],
                        xT_d[128 * i:128 * i + 128, 512 * sc:512 * sc + 512])
                nc.sync.dma_start(cos2t[:, 512 * sc:512 * sc + 512],
                                  cos2_d[:, 512 * sc:512 * sc + 512])
                nc.sync.dma_start(sinm2t[:, 512 * sc:512 * sc + 512],
                                  sinm2_d[:, 512 * sc:512 * sc + 512])
                if sc == 3:
                    for cc in range(4):
                        K = 128 if cc < 3 else 64
                        nc.sync.dma_start(wo[cc][0:K, :],
                                          woT_d[128 * cc:128 * cc + K, :])
            # pre-load the exp table set during the QKV prefix
            nc.scalar.activation(warm[:], biast[:, 0:1], AF.Exp,
                                 bias=0.0, scale=0.0)

            # ---- QKV projection + RoPE emission helpers -------------------
            def qkv_unit(m, sc):
                M, mo = M_SIZES[m], M_OFFS[m]
                t = ps.tile([128, 512], F32, tag="aux", bufs=2,
                            name=f"qkvps{m}_{sc}")
                for h in range(7):
                    nc.tensor.matmul(
                        t[0:M, :], wt[h][:, mo:mo + M],
                        xt[h][:, 512 * sc:512 * sc + 512],
                        start=(h == 0), stop=(h == 6))
                nc.vector.tensor_scalar_add(
                    qkv[m][0:M, 512 * sc:512 * sc + 512], t[0:M, :],
                    biast[0:M, m:m + 1])

            def rope_chunk(m, c):
                # RoPE on a [128, 512] column window of qkv[m].  Window-0
                # moves go out on the (idle) scalar-engine DMA trigger so
                # they don't queue behind the bulk input loads.
                eng = nc.sync
                cs = slice(512 * c, 512 * c + 512)
                xsw = pw.tile([128, 512], BF16, tag="xsw", bufs=2,
                              name=f"xsw{m}_{c}")
                eng.dma_start(xsw[0:32, :], qkv[m][32:64, cs])
                eng.dma_start(xsw[32:64, :], qkv[m][0:32, cs])
                eng.dma_start(xsw[64:96, :], qkv[m][96:128, cs])
                eng.dma_start(xsw[96:128, :], qkv[m][64:96, cs])
                tsin = pw.tile([128, 512], BF16, tag="tsin", bufs=2,
                               name=f"tsin{m}_{c}")
                nc.vector.tensor_tensor(tsin[:], xsw[:], sinm2t[:, cs],
                                        ALU.mult)
                nc.vector.tensor_tensor(qkv[m][:, cs], qkv[m][:, cs],
                                        cos2t[:, cs], ALU.mult)
                nc.vector.tensor_tensor(qkv[m][:, cs], qkv[m][:, cs],
                                        tsin[:], ALU.add)

            def kv_chunk(c):
                # rope the K tile window, duplicate it into k2/q6d, and
                # transpose the V window into v_sb chunks 4c..4c+3
                eng = nc.sync
                cs = slice(512 * c, 512 * c + 512)
                rope_chunk(3, c)
                eng.dma_start(k2[0:64, cs], qkv[3][64:128, cs])
                eng.dma_start(k2[64:128, cs], qkv[3][64:128, cs])
                eng.dma_start(q6d[64:128, cs], qkv[3][0:64, cs])

            def v_chunk(c):
                for i in range(4 * c, 4 * c + 4):
                    t = ps.tile([128, 64], BF16, tag="aux", bufs=2,
                                name=f"vtr{i}")
                    nc.tensor.transpose(
                        t[:], qkv[4][0:64, 128 * i:128 * i + 128], ident[:])
                    nc.vector.tensor_copy(v_sb[i][:, 0:64], t[:])
                    nc.gpsimd.memset(v_sb[i][:, 64:65], 1.0)

            # ---- prefix: only window 0 of K and V, so attention j=0 can
            # start after ~2 QKV units ---------------------------------------
            qkv_unit(3, 0)
            kv_chunk(0)
            qkv_unit(4, 0)
            v_chunk(0)

            # ---- filler queue for dense PE stream: the rest of QKV + RoPE,
            # chunk-major so window 0 of every q-tile lands first -----------
            fillers = deque()
            emitted = {}
            for m in (0, 1, 2):
                fillers.append((None, lambda m=m: qkv_unit(m, 0)))
                fillers.append((("r", m, 0), lambda m=m: rope_chunk(m, 0)))
            for c in range(1, 4):
                fillers.append((None, lambda c=c: load_wave(c)))
                fillers.append((None, lambda c=c: qkv_unit(3, c)))
                fillers.append((("kv", c), lambda c=c: kv_chunk(c)))
                fillers.append((None, lambda c=c: qkv_unit(4, c)))
                fillers.append((("v", c), lambda c=c: v_chunk(c)))
                for m in (0, 1, 2):
                    fillers.append((None, lambda m=m, c=c: qkv_unit(m, c)))
                    fillers.append((("r", m, c),
                                    lambda m=m, c=c: rope_chunk(m, c)))

            def _pop_one():
                key, fn = fillers.popleft()
                fn()
                if key is not None:
                    emitted[key] = True

            def pop_fillers(n):
                for _ in range(n):
                    if fillers:
                        _pop_one()

            def ensure(key):
                while fillers and not emitted.get(key, False):
                    _pop_one()

            emitted[("kv", 0)] = emitted[("v", 0)] = True

            # ---- attention ------------------------------------------------
            # group = (chunks, widths): full pairs then diagA, diagB
            def groups_for(j):
                gs = []
                for c0 in range(0, 4 * j, 2):
                    gs.append(([c0, c0 + 1], [512, 512]))
                gs.append(([4 * j, 4 * j + 1], [512, 384]))
                gs.append(([4 * j + 2, 4 * j + 3], [256, 128]))
                return gs

            attn = {}   # (hp, j) -> SBUF tile holding normalized attnT
            HP_ORDER = [3, 0, 1, 2]
            dbg_rcp_row = [0]

            def scores_lhs_rhs(hp, h, c, qs):
                # returns (lhsT, rhs) for scores matmul of head h, chunk c
                cs = slice(128 * c, 128 * c + 128)
                if hp < 3:
                    if h % 2 == 0:
                        return k2[0:64, cs], qkv[hp][0:64, qs]
                    return k2[64:128, cs], qkv[hp][64:128, qs]
                # head 6: alternate row groups by chunk parity for PE overlap
                if c % 2 == 0:
                    return k2[0:64, cs], qkv[3][0:64, qs]
                return k2[64:128, cs], q6d[64:128, qs]

            for j in range(4):
                gs = groups_for(j)
                nkc = 4 * j + 4
                ensure(("kv", j))
                ensure(("v", j))
                for hp in HP_ORDER:
                    if hp < 3:
                        ensure(("r", hp, j))
                    heads = [2 * hp, 2 * hp + 1] if hp < 3 else [6]
                    pv = {h: ps.tile([65, 512], F32, tag=f"pv{h % 2}",
                                     name=f"pv{hp}_{j}_{h}")
                          for h in heads}
                    pending = None
                    for gi, (chunks, widths) in enumerate(gs):
                        W = sum(widths)
                        offs = [0, widths[0]]
                        scts, prs = {}, {}
                        for h in heads:
                            strm = (h % 2) if hp < 3 else (gi % 2)
                            sct = ps.tile([128, W], F32, tag=f"sc{strm}",
                                          name=f"sc{hp}_{j}_{gi}_{h}")
                            scts[h] = sct
                            for i, c in enumerate(chunks):
                                w = widths[i]
                                qs = slice(512 * j + 512 - w, 512 * j + 512)
                                if hp == 3 and gi == len(gs) - 1:
                                    # diagB solo head: both chunks in one
                                    # bank -> keep on one row group
                                    cs = slice(128 * c, 128 * c + 128)
                                    lhs, rhs = k2[0:64, cs], qkv[3][0:64, qs]
                                else:
                                    lhs, rhs = scores_lhs_rhs(hp, h, c, qs)
                                nc.tensor.matmul(
                                    sct[:, offs[i]:offs[i] + w], lhs, rhs,
                                    start=True, stop=True)
                        for h in heads:
                            strm = (h % 2) if hp < 3 else (gi % 2)
                            pt = pw.tile([128, W], BF16, tag=f"pr{strm}",
                                         bufs=3, name=f"pr{hp}_{j}_{gi}_{h}")
                            prs[h] = pt
                            nc.scalar.activation(pt[:, 0:W], scts[h][:, 0:W],
                                                 AF.Exp, bias=0.0, scale=0.125)
                            # zero the above-diagonal triangles of diag chunks
                            for i, c in enumerate(chunks):
                                t = c - 4 * j
                                if t >= 0:
                                    sl = pt[:, offs[i]:offs[i] + 128]
                                    nc.gpsimd.affine_select(
                                        out=sl, in_=sl, compare_op=ALU.is_ge,
                                        fill=0.0, base=0, pattern=[[1, 128]],
                                        channel_multiplier=-1)
                            if DEBUG and j == 0:
                                h_ = heads.index(h) if hp == 3 else h
                                co = 0 if gi == len(gs) - 2 else 896
                                nc.sync.dma_start(
                                    dbg["dpr"][128 * h:128 * h + 128,
                                               co:co + W], pt[:, 0:W])
                        if pending is not None:
                            pending()
                        def make_pv(chunks=chunks, widths=widths, offs=offs,
                                    prs=prs):
                            for h in heads:
                                for i, c in enumerate(chunks):
                                    w = widths[i]
                                    nc.tensor.matmul(
                                        pv[h][:, 512 - w:512], v_sb[c][:],
                                        prs[h][:, offs[i]:offs[i] + w],
                                        start=(c == 0), stop=(c == nkc - 1))
                        pending = make_pv
                        pop_fillers(1)
                    pending()
                    # normalize: evacuate pv to SBUF promptly (frees the PSUM
                    # bank for the next head pair), then 1/rowsum via fast
                    # approx + broadcast + in-place scale on SBUF
                    for h in heads:
                        if (hp, j) not in attn:
                            P = 128 if hp < 3 else 64
                            attn[(hp, j)] = pw.tile(
                                [P, 512], BF16, tag=f"attn{hp}", bufs=2,
                                name=f"attn{hp}_{j}")
                        dst = attn[(hp, j)][64 * (h % 2):64 * (h % 2) + 64, :]
                        au = pw.tile([64, 512], BF16, tag=f"au{h % 2}",
                                     bufs=2, name=f"au{hp}_{j}_{h}")
                        rs = pw.tile([1, 512], F32, tag="rs", bufs=2,
                                     name=f"rs{hp}_{j}_{h}")
                        # evacuate pv PSUM with high scheduler priority: the
                        # bank is the next head-pair's PV accumulator
                        with tc.high_priority():
                            nc.vector.tensor_copy(au[:], pv[h][0:64, :])
                            # custom-DVE ops drop the input partition offset,
                            # so stage the rowsum row to partition 0 first
                            nc.vector.tensor_copy(rs[:], pv[h][64:65, :])
                        rcp = pw.tile([1, 512], F32, tag="rcp", bufs=2,
                                      name=f"rcp{hp}_{j}_{h}")
                        nc.vector.reciprocal_approx_fast(rcp[:], rs[:])
                        if DEBUG:
                            r = dbg_rcp_row[0]
                            dbg_rcp_row[0] += 1
                            nc.sync.dma_start(dbg["drcp"][r:r + 1, :], rcp[:])
                        rb = pw.tile([64, 512], F32, tag="rb", bufs=2,
                                     name=f"rb{hp}_{j}_{h}")
                        nc.gpsimd.partition_broadcast(rb[:], rcp[:])
                        nc.vector.tensor_tensor(dst, au[:], rb[:], ALU.mult)
                    if DEBUG:
                        P = 128 if hp < 3 else 64
                        nc.sync.dma_start(
                            dbg["dattn"][128 * hp:128 * hp + P,
                                         512 * j:512 * j + 512],
                            attn[(hp, j)][0:P, :])
                # queue o_proj units for this j as fillers for the next j
                def oproj_unit(j=j, ot=0):
                    pys = ps.tile([128, 512], F32, tag="aux", bufs=2,
                                  name=f"py{j}_{ot}")
                    for cc in range(4):
                        K = 128 if cc < 3 else 64
                        nc.tensor.matmul(
                            pys[:], wo[cc][0:K, 128 * ot:128 * ot + 128],
                            attn[(cc, j)][0:K, :],
                            start=(cc == 0), stop=(cc == 3))
                    osb = pw.tile([128, 512], F32, tag="osb", bufs=2,
                                  name=f"osb{j}_{ot}")
                    with tc.high_priority():
                        nc.vector.tensor_copy(osb[:], pys[:])
                    nc.sync.dma_start(
                        yT_d[128 * ot:128 * ot + 128,
                             512 * j:512 * j + 512], osb[:])
                for ot in range(7):
                    fillers.append((None, lambda j=j, ot=ot: oproj_unit(j, ot)))
            # flush remaining o_proj units (last j's)
            while fillers:
                fillers.popleft()[1]()

            if DEBUG:
                for m in range(5):
                    nc.sync.dma_start(dbg["dqkv"][128 * m:128 * m + 128, :],
                                      qkv[m][:])
                nc.sync.dma_start(dbg["dk2"][:], k2[:])
                nc.sync.dma_start(dbg["dq6d"][:], q6d[:])
                for i in range(16):
                    nc.sync.dma_start(dbg["dv"][128 * i:128 * i + 128, :],
                                      v_sb[i][:])

    nc.compile()
    return nc


def _host_prep(inputs):
    import ml_dtypes
    bf16 = ml_dtypes.bfloat16
    hid = np.ascontiguousarray(np.asarray(inputs["hidden_states"], np.float32))
    pos = np.asarray(inputs["position_ids"])[0].astype(np.float32)
    Wq = np.asarray(inputs["Wq"], np.float32)
    bq = np.asarray(inputs["bq"], np.float32)
    Wk = np.asarray(inputs["Wk"], np.float32)
    bk = np.asarray(inputs["bk"], np.float32)
    Wv = np.asarray(inputs["Wv"], np.float32)
    bv = np.asarray(inputs["bv"], np.float32)
    Wo = np.asarray(inputs["Wo"], np.float32)

    inv = (1.0 / (ROPE_THETA ** (np.arange(0, HD, 2, dtype=np.float32) / HD))
           ).astype(np.float32)
    freqs = pos[:, None] * inv[None, :]
    emb = np.concatenate([freqs, freqs], -1)            # [S, 64]
    cosT = np.cos(emb).T.astype(np.float32)             # [64, S]
    sinT = np.sin(emb).T.astype(np.float32)
    sinm = sinT.copy()
    sinm[0:32] *= -1.0                                  # fold rotate_half sign
    cos2 = np.ascontiguousarray(np.vstack([cosT, cosT])).astype(bf16)
    sinm2 = np.ascontiguousarray(np.vstack([sinm, sinm])).astype(bf16)

    maps = []
    for b in range(B):
        for g in range(2):
            xT = np.ascontiguousarray(hid[b].T).astype(bf16)
            Wsl = np.concatenate([Wq[448 * g:448 * g + 448],
                                  Wk[64 * g:64 * g + 64],
                                  Wv[64 * g:64 * g + 64]], 0)
            wT = np.ascontiguousarray(Wsl.T).astype(bf16)  # [896, 576]
            bias = np.zeros(640, np.float32)
            bias[:576] = np.concatenate([bq[448 * g:448 * g + 448],
                                         bk[64 * g:64 * g + 64],
                                         bv[64 * g:64 * g + 64]])
            woT = np.ascontiguousarray(Wo[:, 448 * g:448 * g + 448].T
                                       ).astype(bf16)
            maps.append(dict(xT=xT, wT=wT, bias=bias, woT=woT,
                             cos2=cos2, sinm2=sinm2,
                             ident64=np.eye(64, dtype=bf16)))
    return maps


def kernel(**inputs) -> np.ndarray:
    from concourse.bass_utils import run_bass_kernel_spmd

    if "nc" not in _PROGRAM_CACHE:
        _PROGRAM_CACHE["nc"] = _build_program()
    nc = _PROGRAM_CACHE["nc"]

    in_maps = _host_prep(inputs)
    res = run_bass_kernel_spmd(nc, in_maps, core_ids=list(range(8)),
                               **_PROGRAM_CACHE.get("run_kwargs", {}))
    _PROGRAM_CACHE["last_result"] = res
    yTs = [res.results[i]["yT"] for i in range(8)]
    out = np.stack([(yTs[2 * b] + yTs[2 * b + 1]).T for b in range(B)], 0)
    return np.ascontiguousarray(out)
